# revision 14
# baseline (speedup 1.0000x reference)
"""MultiHeadAttention (B=4,T=2048,D=2048,NQ=16,NK=8,H=128) on 8 trn2 cores.

Sharding: core c -> batch b=c//2, half=c%2. Each core computes the partial
output for batch b restricted to q-heads [half*8, half*8+8) (kv-heads
[half*4, half*4+4)); host sums the two partials per batch (o_proj
contraction over heads is split across the core pair).
"""
import numpy as np
import concourse.bass as bass
import concourse.tile as tile
from concourse import bacc, mybir
from concourse import bass_utils

B, T, D = 4, 2048, 2048
NQ, NK, H = 16, 8, 128
NH, NKV = 8, 4          # per-core q heads / kv heads
THETA = 10000.0
EPS = 1e-6
TCH = 256               # projection-pass T chunk
NCH = T // TCH
NDK = D // 128
QCH = 512               # attention q chunk
NQC = T // QCH
NTB = T // 128

f32 = mybir.dt.float32
f32r = mybir.dt.float32r
AF = mybir.ActivationFunctionType

TRACE = False
LAST_EXEC_NS = None
_CACHE = {}


def _install_hook():
    import contextlib, ctypes, sys, types
    if "antenv.axon_hooks" in sys.modules:
        return
    lib = ctypes.CDLL("/opt/axon/libaxon_pjrt.so")
    lib.axon_start_nrt_profile.argtypes = [ctypes.POINTER(ctypes.c_int64), ctypes.c_size_t]
    lib.axon_start_nrt_profile.restype = ctypes.c_int64
    lib.axon_stop_nrt_profile.argtypes = [ctypes.c_char_p]
    lib.axon_stop_nrt_profile.restype = ctypes.c_int64

    @contextlib.contextmanager
    def _hook(output_dir, device_ids):
        import jax
        jax.devices()
        ids = (ctypes.c_int64 * len(device_ids))(*device_ids) if device_ids else None
        rc = lib.axon_start_nrt_profile(ids, len(device_ids) if device_ids else 0)
        if rc != 0:
            raise RuntimeError(f"axon_start_nrt_profile rc={rc}")
        try:
            yield
        finally:
            n = lib.axon_stop_nrt_profile(str(output_dir).encode())
            if n < 0:
                raise RuntimeError(f"axon_stop_nrt_profile rc={n}")

    mod = types.ModuleType("antenv.axon_hooks")
    mod.get_axon_ntff_profile_hook = lambda: _hook
    mod.set_axon_ntff_profile_hook = lambda h: None
    sys.modules["antenv.axon_hooks"] = mod
    bass_utils.upload_artifacts = lambda tmpdir: "local://" + str(tmpdir)


def _build():
    nc = bacc.Bacc("TRN2", target_bir_lowering=False, debug=False, num_devices=8)
    x_ap = nc.dram_tensor("x", [T, D], f32r, kind="ExternalInput").ap()
    wq_ap = nc.dram_tensor("wq", [128, NH * NDK * 128], f32r, kind="ExternalInput").ap()
    wk_ap = nc.dram_tensor("wk", [128, NKV * NDK * 128], f32r, kind="ExternalInput").ap()
    wv_ap = nc.dram_tensor("wv", [128, NKV * NDK * 128], f32r, kind="ExternalInput").ap()
    wo_ap = nc.dram_tensor("wo", [128, NH * D], f32r, kind="ExternalInput").ap()
    cs_ap = nc.dram_tensor("cs", [128, T], f32, kind="ExternalInput").ap()
    sn_ap = nc.dram_tensor("sn", [128, T], f32, kind="ExternalInput").ap()
    qsc_ap = nc.dram_tensor("qsc", [128, 1], f32, kind="ExternalInput").ap()
    ksc_ap = nc.dram_tensor("ksc", [128, 1], f32, kind="ExternalInput").ap()
    cm_ap = nc.dram_tensor("cm", [128, 4 * QCH], f32r, kind="ExternalInput").ap()
    id_ap = nc.dram_tensor("ident", [128, 128], f32r, kind="ExternalInput").ap()
    oc_ap = nc.dram_tensor("onesc", [128, 1], f32r, kind="ExternalInput").ap()
    out_ap = nc.dram_tensor("out", [T, D], f32, kind="ExternalOutput").ap()

    with tile.TileContext(nc) as tc:
        with tc.tile_pool(name="perm", bufs=1) as perm, \
             tc.tile_pool(name="psum", bufs=1, space="PSUM") as pp, \
             tc.tile_pool(name="dram", bufs=1, space="DRAM") as dpool:
            ident = perm.tile([128, 128], f32r)
            nc.sync.dma_start(ident[:], id_ap[:])
            ones_col = perm.tile([128, 1], f32r)
            nc.sync.dma_start(ones_col[:], oc_ap[:])
            ones_row = perm.tile([1, 128], f32)
            nc.vector.memset(ones_row[:], 1.0)
            qsc_t = perm.tile([128, 1], f32)
            nc.sync.dma_start(qsc_t[:], qsc_ap[:])
            ksc_t = perm.tile([128, 1], f32)
            nc.sync.dma_start(ksc_t[:], ksc_ap[:])
            cm_t = perm.tile([128, 4 * QCH], f32r)
            nc.sync.dma_start(cm_t[:], cm_ap[:])
            eps_t = perm.tile([1, 1], f32)
            nc.vector.memset(eps_t[:], EPS)
            qt_sp = dpool.tile([128, NH * T], f32r)

            def build_xts(pool, ch):
                t0 = ch * TCH
                xin = []
                for j in range(2):
                    xj = pool.tile([128, D], f32r, tag=f"xin{j}")
                    nc.sync.dma_start(xj[:], x_ap[t0 + j * 128: t0 + (j + 1) * 128, :])
                    xin.append(xj)
                xts = pool.tile([128, NDK * TCH], f32r, tag="xts")
                for dk in range(NDK):
                    trp = pp.tile([128, 512], f32r, tag="tr", bufs=2)
                    for j in range(2):
                        nc.tensor.transpose(
                            trp[:, j * 128:(j + 1) * 128],
                            xin[j][:, dk * 128:(dk + 1) * 128], ident[:])
                    nc.vector.tensor_copy(xts[:, dk * TCH:(dk + 1) * TCH], trp[:, 0:TCH])
                cs_c = pool.tile([128, TCH], f32, tag="csc")
                nc.sync.dma_start(cs_c[:], cs_ap[:, t0:t0 + TCH])
                sn_c = pool.tile([128, TCH], f32, tag="snc")
                nc.sync.dma_start(sn_c[:], sn_ap[:, t0:t0 + TCH])
                return xts, cs_c, sn_c

            def drain(pool, acc, sc_t, cs_c, sn_c, dst):
                """acc: psum (128,TCH) f32 -> RMSNorm*(gain) + RoPE -> dst f32r."""
                tmp = pool.tile([128, TCH], f32, tag="dtmp")
                nc.vector.tensor_copy(tmp[:], acc)
                sq = pool.tile([128, TCH], f32r, tag="dsq")
                nc.scalar.activation(sq[:], tmp[:], AF.Square)
                row = pp.tile([1, 512], f32, tag="row")
                nc.tensor.matmul(row[:, 0:TCH], ones_col[:], sq[:], start=True, stop=True)
                lrow = pool.tile([1, TCH], f32, tag="dlrow")
                nc.scalar.activation(lrow[:], row[:, 0:TCH], AF.Ln, bias=eps_t[:], scale=1.0 / H)
                rstd = pool.tile([1, TCH], f32, tag="drstd")
                nc.scalar.activation(rstd[:], lrow[:], AF.Exp, scale=-0.5)
                bc = pp.tile([128, 512], f32, tag="bc")
                nc.tensor.matmul(bc[:, 0:TCH], ones_row[:], rstd[:], start=True, stop=True)
                qn = pool.tile([128, TCH], f32, tag="dqn")
                nc.vector.tensor_mul(qn[:], tmp[:], bc[:, 0:TCH])
                nc.vector.tensor_scalar_mul(qn[:], qn[:], sc_t[:])
                qsw = pool.tile([128, TCH], f32, tag="dqsw")
                nc.sync.dma_start(qsw[0:64, :], qn[64:128, :])
                nc.sync.dma_start(qsw[64:128, :], qn[0:64, :])
                ta = pool.tile([128, TCH], f32, tag="dta")
                nc.vector.tensor_mul(ta[:], qn[:], cs_c[:])
                tb = pool.tile([128, TCH], f32, tag="dtb")
                nc.vector.tensor_mul(tb[:], qsw[:], sn_c[:])
                nc.vector.tensor_add(dst, ta[:], tb[:])

            # ---- pass A: q projection -> norm/rope -> DRAM spill ----
            with tc.tile_pool(name="pa", bufs=1) as pa:
                wq_t = pa.tile([128, NH * NDK * 128], f32r)
                nc.sync.dma_start(wq_t[:], wq_ap[:])
                for ch in range(NCH):
                    t0 = ch * TCH
                    xts, cs_c, sn_c = build_xts(pa, ch)
                    for g in range(2):
                        accs = [pp.tile([128, 512], f32, tag=f"acc{i}", name=f"acc{i}") for i in range(4)]
                        for dk in range(NDK):
                            for hh in range(4):
                                h = g * 4 + hh
                                nc.tensor.matmul(
                                    accs[hh][:, 0:TCH],
                                    wq_t[:, (h * NDK + dk) * 128:(h * NDK + dk + 1) * 128],
                                    xts[:, dk * TCH:(dk + 1) * TCH],
                                    start=(dk == 0), stop=(dk == NDK - 1))
                        for hh in range(4):
                            h = g * 4 + hh
                            stg = pa.tile([128, TCH], f32r, tag="stg", bufs=2)
                            drain(pa, accs[hh][:, 0:TCH], qsc_t, cs_c, sn_c, stg[:])
                            nc.sync.dma_start(qt_sp[:, h * T + t0: h * T + t0 + TCH], stg[:])

            # ---- pass B: k,v projection; k -> kT resident, v -> vT resident ----
            with tc.tile_pool(name="kvp", bufs=1) as kvpool:
                kT = kvpool.tile([128, NKV * T], f32r)
                vT = kvpool.tile([128, NKV * T], f32r)
                with tc.tile_pool(name="pb", bufs=1) as pb:
                    wk_t = pb.tile([128, NKV * NDK * 128], f32r)
                    nc.sync.dma_start(wk_t[:], wk_ap[:])
                    wv_t = pb.tile([128, NKV * NDK * 128], f32r)
                    nc.sync.dma_start(wv_t[:], wv_ap[:])
                    for ch in range(NCH):
                        t0 = ch * TCH
                        xts, cs_c, sn_c = build_xts(pb, ch)
                        # k group
                        accs = [pp.tile([128, 512], f32, tag=f"acc{i}", name=f"acc{i}") for i in range(4)]
                        for dk in range(NDK):
                            for kv in range(NKV):
                                nc.tensor.matmul(
                                    accs[kv][:, 0:TCH],
                                    wk_t[:, (kv * NDK + dk) * 128:(kv * NDK + dk + 1) * 128],
                                    xts[:, dk * TCH:(dk + 1) * TCH],
                                    start=(dk == 0), stop=(dk == NDK - 1))
                        for kv in range(NKV):
                            drain(pb, accs[kv][:, 0:TCH], ksc_t, cs_c, sn_c,
                                  kT[:, kv * T + t0: kv * T + t0 + TCH])
                        # v group
                        accs = [pp.tile([128, 512], f32, tag=f"acc{i}", name=f"acc{i}") for i in range(4)]
                        for dk in range(NDK):
                            for kv in range(NKV):
                                nc.tensor.matmul(
                                    accs[kv][:, 0:TCH],
                                    wv_t[:, (kv * NDK + dk) * 128:(kv * NDK + dk + 1) * 128],
                                    xts[:, dk * TCH:(dk + 1) * TCH],
                                    start=(dk == 0), stop=(dk == NDK - 1))
                        for kv in range(NKV):
                            vtmp = pb.tile([128, TCH], f32r, tag="vtmp")
                            nc.vector.tensor_copy(vtmp[:], accs[kv][:, 0:TCH])
                            trp = pp.tile([128, 512], f32r, tag="tr", bufs=2)
                            for j in range(2):
                                nc.tensor.transpose(
                                    trp[:, j * 128:(j + 1) * 128],
                                    vtmp[:, j * 128:(j + 1) * 128], ident[:])
                            nc.vector.tensor_copy(
                                vT[:, kv * T + t0: kv * T + t0 + TCH], trp[:, 0:TCH])

                # ---- attention: softmax(q k^T) v, causal, no max-subtraction ----
                with tc.tile_pool(name="ap2", bufs=1) as ap2:
                    attn = ap2.tile([128, NH * T], f32r)
                    with tc.tile_pool(name="at", bufs=1) as at:
                        for h in range(NH):
                            kv = h // 2
                            qh = at.tile([128, T], f32r, tag="qh", bufs=2)
                            nc.sync.dma_start(qh[:], qt_sp[:, h * T:(h + 1) * T])
                            for qi in range(NQC):
                                o_ps = pp.tile([128, 512], f32, tag="acc2")
                                acc_sb = at.tile([128, QCH], f32r, tag="asb")
                                nkj = 4 * qi + 4
                                for kj in range(nkj):
                                    s_ps = pp.tile([128, 512], f32, tag=f"acc{kj % 2 and 1 or 0}")
                                    nc.tensor.matmul(
                                        s_ps[:], kT[:, kv * T + kj * 128: kv * T + (kj + 1) * 128],
                                        qh[:, qi * QCH:(qi + 1) * QCH], start=True, stop=True)
                                    pt = at.tile([128, QCH], f32r, tag="pt", bufs=3)
                                    nc.scalar.activation(pt[:], s_ps[:], AF.Exp)
                                    m = kj - 4 * qi
                                    if m >= 0:
                                        nc.vector.tensor_mul(pt[:], pt[:], cm_t[:, m * QCH:(m + 1) * QCH])
                                    if kj == 0:
                                        nc.vector.tensor_copy(acc_sb[:], pt[:])
                                    else:
                                        nc.vector.tensor_add(acc_sb[:], acc_sb[:], pt[:])
                                    nc.tensor.matmul(
                                        o_ps[:], vT[:, kv * T + kj * 128: kv * T + (kj + 1) * 128],
                                        pt[:], start=(kj == 0), stop=(kj == nkj - 1))
                                row = pp.tile([1, 512], f32, tag="row")
                                nc.tensor.matmul(row[:], ones_col[:], acc_sb[:], start=True, stop=True)
                                lr = at.tile([1, QCH], f32, tag="alr")
                                nc.scalar.activation(lr[:], row[:], AF.Ln)
                                rrow = at.tile([1, QCH], f32, tag="arr")
                                nc.scalar.activation(rrow[:], lr[:], AF.Exp, scale=-1.0)
                                bc = pp.tile([128, 512], f32, tag="bc")
                                nc.tensor.matmul(bc[:], ones_row[:], rrow[:], start=True, stop=True)
                                o_sb = at.tile([128, QCH], f32, tag="osb")
                                nc.vector.tensor_copy(o_sb[:], o_ps[:])
                                nc.vector.tensor_mul(
                                    attn[:, h * T + qi * QCH: h * T + (qi + 1) * QCH],
                                    o_sb[:], bc[:])

                    # ---- o_proj partial: out[tc,dc] = sum_h attnT_h^T @ wo_h ----
                    with tc.tile_pool(name="op", bufs=1) as opool:
                        for dc in range(4):
                            wos = []
                            for h in range(NH):
                                w = opool.tile([128, 512], f32r, tag="wo", bufs=8)
                                nc.sync.dma_start(w[:], wo_ap[:, h * D + dc * 512: h * D + (dc + 1) * 512])
                                wos.append(w)
                            for ti in range(NTB):
                                ops = pp.tile([128, 512], f32, tag=f"acc{ti % 2}")
                                for h in range(NH):
                                    nc.tensor.matmul(
                                        ops[:], attn[:, h * T + ti * 128: h * T + (ti + 1) * 128],
                                        wos[h][:], start=(h == 0), stop=(h == NH - 1))
                                stg = opool.tile([128, 512], f32, tag="ostg", bufs=2)
                                nc.vector.tensor_copy(stg[:], ops[:])
                                nc.sync.dma_start(
                                    out_ap[ti * 128:(ti + 1) * 128, dc * 512:(dc + 1) * 512], stg[:])

    nc.compile()
    return nc


def _pack(w):
    """(nh, D, H) -> (128, nh*NDK*128): col block (h*NDK+dk)*128 = w[h, dk*128:+128, :]."""
    nh = w.shape[0]
    a = w.reshape(nh, NDK, 128, H).transpose(2, 0, 1, 3)
    return np.ascontiguousarray(a.reshape(128, nh * NDK * H))


def _numpy_ref(x, mask, position, qp, kvp, op, qns, kns):
    def rms(v, s):
        var = (v * v).mean(-1, keepdims=True)
        return v / np.sqrt(var + EPS) * (1.0 + s)

    def rope(v, pos):
        ts = THETA ** (np.arange(64, dtype=np.float32) * 2.0 / H)
        ang = pos.astype(np.float32)[:, :, None, None] / ts
        sn, cs = np.sin(ang), np.cos(ang)
        x1, x2 = v[..., :64], v[..., 64:]
        return np.concatenate([x1 * cs - x2 * sn, x2 * cs + x1 * sn], -1)

    q = np.einsum('BTD,NDH->BTNH', x, qp)
    k = np.einsum('BTD,KDH->BTKH', x, kvp[0])
    v = np.einsum('BTD,KDH->BTKH', x, kvp[1])
    q = rope(rms(q, qns), position) * (H ** -0.5)
    k = rope(rms(k, kns), position)
    q = q.transpose(0, 2, 1, 3)
    k = np.repeat(k.transpose(0, 2, 1, 3), NQ // NK, 1)
    v = np.repeat(v.transpose(0, 2, 1, 3), NQ // NK, 1)
    s = np.einsum('BHtD,BHTD->BHtT', q, k) / np.sqrt(np.float32(H))
    s = np.where(mask[:, None], s, np.float32(-2.3819763e+38))
    s = s - s.max(-1, keepdims=True)
    w = np.exp(s)
    w /= w.sum(-1, keepdims=True)
    o = np.einsum('BHtT,BHTD->BHtD', w, v)
    return np.einsum('BNTH,NHD->BTD', o, op).astype(np.float32)


def kernel(**inputs):
    global LAST_EXEC_NS
    x = np.asarray(inputs["x"], np.float32)
    mask = np.asarray(inputs["mask"])
    position = np.asarray(inputs["position"])
    qp = np.asarray(inputs["q_proj"], np.float32)
    kvp = np.asarray(inputs["kv_proj"], np.float32)
    op = np.asarray(inputs["o_proj"], np.float32)
    qns = np.asarray(inputs["q_norm_scale"], np.float32)
    kns = np.asarray(inputs["k_norm_scale"], np.float32)

    tril = np.tril(np.ones((T, T), bool))
    if mask.shape != (B, T, T) or not all(np.array_equal(mask[b], tril) for b in range(B)):
        return _numpy_ref(x, mask, position, qp, kvp, op, qns, kns)

    if "nc" not in _CACHE:
        _CACHE["nc"] = _build()
    nc = _CACHE["nc"]

    halves = []
    for half in range(2):
        halves.append((
            _pack(qp[half * NH:(half + 1) * NH]),
            _pack(kvp[0, half * NKV:(half + 1) * NKV]),
            _pack(kvp[1, half * NKV:(half + 1) * NKV]),
            np.ascontiguousarray(
                op[half * NH:(half + 1) * NH].transpose(1, 0, 2).reshape(128, NH * D)),
        ))
    qsc = ((1.0 + qns) / H).reshape(128, 1).astype(np.float32)
    ksc = (1.0 + kns).reshape(128, 1).astype(np.float32)
    ts = THETA ** (np.arange(64, dtype=np.float64) * 2.0 / H)
    fidx = np.arange(QCH)[None, :]
    pidx = np.arange(128)[:, None]
    cm = np.concatenate(
        [(fidx >= m * 128 + pidx).astype(np.float32) for m in range(4)], axis=1)
    cm = np.ascontiguousarray(cm)

    in_maps = []
    for c in range(8):
        b, half = c // 2, c % 2
        wq, wk, wv, wo = halves[half]
        ang = position[b].astype(np.float64)[None, :] / ts[:, None]
        sn = np.sin(ang).astype(np.float32)
        cs = np.cos(ang).astype(np.float32)
        in_maps.append({
            "x": np.ascontiguousarray(x[b]),
            "wq": wq, "wk": wk, "wv": wv, "wo": wo,
            "cs": np.ascontiguousarray(np.concatenate([cs, cs], 0)),
            "sn": np.ascontiguousarray(np.concatenate([-sn, sn], 0)),
            "qsc": qsc, "ksc": ksc, "cm": cm,
            "ident": np.eye(128, dtype=np.float32),
            "onesc": np.ones((128, 1), np.float32),
        })

    if TRACE:
        _install_hook()
    last_err = None
    for _ in range(3):
        try:
            res = bass_utils.run_bass_kernel_spmd(nc, in_maps, list(range(8)), trace=TRACE)
            break
        except Exception as e:  # transient NRT device wedge
            last_err = e
    else:
        raise last_err
    LAST_EXEC_NS = getattr(res, "exec_time_ns", None)

    out = np.empty((B, T, D), np.float32)
    for b in range(B):
        out[b] = res.results[2 * b]["out"] + res.results[2 * b + 1]["out"]
    return out

